# revision 43
# baseline (speedup 1.0000x reference)
"""Bahdanau additive attention kernel for Trainium2 (8 NeuronCores).

Problem shapes (hardcoded): B=4, Q=256, V=2048, H=512, U=128, fp32.

reference:
    pq = queries @ w1                  # [B,Q,U]
    pv = values  @ w2                  # [B,V,U]
    scores[b,q,v] = sum_u tanh(pq[b,q,u] + pv[b,v,u]) * v[u]
    attn = softmax(scores, axis=-1)
    out  = attn @ values               # [B,Q,H]

Sharding: 8 cores = 4 batches x 2 query-halves. Each core handles a full
softmax over V for its [128, H] query slice -> no collectives needed.

Per-core dataflow (ACT-roofline design: the 128*2048*128 tanh evals are
the hard floor -- ScalarE runs them at 1 elem/lane/cycle regardless of
dtype -- so everything else is arranged to hide underneath them):
  - pqT [U, Qloc] and pvT [U, V] via PE projections; the host supplies
    transposed queries/values (layout-only prep) so no on-chip
    transposes sit on the critical path. pv matmuls run in float32r
    (1.5 cyc/row) pipelined behind the chunked valsT DMAs.
  - 16 iterations of 8 q's (two matvec waves of 4 PE col-strips each):
      DVE: 8 per-partition adds  pvT + pqT[:,q]  (fp16, 2x mode)
      ACT: one merged tanh over [128, 8*2048] fp16 (amortizes the
           224-cycle per-instruction overhead 8x)
      PE : col-tiled matvecs (tile_position=(0,32s), shared M=32
           stationary window with v at window-col g) accumulate score
           rows for 4 q's concurrently into PSUM strips.
  - softmax: exp without max-subtract (|scores| <= sum|v| ~ 9, safe in
    fp32) quartered for overlap, accum_out gives row sums for free,
    DVE reciprocal.
  - out = (eT @ values) * 1/sum: 16 PE transposes of e (fp16) + 16
    accumulating fp16 matmuls against host-supplied fp16 values tiles,
    overlapped with the exp quarters via a nested PSUM pool.
"""

from contextlib import ExitStack

import numpy as np

import concourse.bacc as bacc
import concourse.tile as tile
from concourse import mybir

B, Q, V, H, U = 4, 256, 2048, 512, 128
QL = Q // 2            # per-core queries
VT = V // 128          # 16 value tiles
HT = H // 128          # 4 hidden tiles
NB = V // 512          # 4 psum bank chunks of the scores row

F32 = mybir.dt.float32
F16 = mybir.dt.float16


def build_nc(t_dtype=F16):
    nc = bacc.Bacc("TRN2", target_bir_lowering=False, debug=False)
    F32R = mybir.dt.float32r
    qT_ext = nc.declare_dram_parameter("qT", [HT, 128, QL], F32, isOutput=False)
    valsT_ext = nc.declare_dram_parameter(
        "valsT", [NB, HT, 128, 512], F32R, isOutput=False)
    vals16_ext = nc.declare_dram_parameter("vals16", [VT, 128, H], F16, isOutput=False)
    w1_ext = nc.declare_dram_parameter("w1", [HT, 128, U], F32, isOutput=False)
    w2_ext = nc.declare_dram_parameter("w2", [HT, 128, U], F32R, isOutput=False)
    id_ext = nc.declare_dram_parameter("identity16", [128, 128], F16, isOutput=False)
    vpad_ext = nc.declare_dram_parameter("vpad", [128, 64], F16, isOutput=False)
    out_ext = nc.declare_dram_parameter("out", [QL, H], F32, isOutput=True)

    with tile.TileContext(nc) as tc, ExitStack() as ctx:
        singles = ctx.enter_context(tc.tile_pool(name="singles", bufs=1))
        work = ctx.enter_context(tc.tile_pool(name="work", bufs=3))
        apool = ctx.enter_context(tc.tile_pool(name="adds", bufs=2))
        tpool = ctx.enter_context(tc.tile_pool(name="tanh", bufs=2))

        # --- inputs; one dma_start per tensor (a single DMA already
        # fans out over all 16 SDMA engines). valsT arrives in 4 v-chunks
        # so the pv build can pipeline behind the transfers. ------------
        sb_valsT = singles.tile([128, NB, HT, 512], F32R)
        # Chunk 0 arrives as two 256-col halves so the pv build (and with
        # it the whole tanh ramp) starts at half the first-chunk latency.
        for h in range(2):
            nc.sync.dma_start(
                out=sb_valsT[:, 0, :, h * 256:(h + 1) * 256],
                in_=valsT_ext[0].rearrange("t p j -> p t j")[:, :, h * 256:(h + 1) * 256])
        sb_w2 = singles.tile([128, HT, U], F32R)
        nc.sync.dma_start(out=sb_w2, in_=w2_ext.rearrange("t p u -> p t u"))
        sb_w1 = singles.tile([128, HT, U], F32)
        nc.sync.dma_start(out=sb_w1, in_=w1_ext.rearrange("t p u -> p t u"))
        sb_qT = singles.tile([128, HT, QL], F32)
        nc.sync.dma_start(out=sb_qT, in_=qT_ext.rearrange("t p q -> p t q"))
        for c in range(1, NB):
            nc.sync.dma_start(
                out=sb_valsT[:, c, :, :],
                in_=valsT_ext[c].rearrange("t p j -> p t j"))
        sb_vals16 = singles.tile([128, VT, H], F16)
        nc.sync.dma_start(out=sb_vals16, in_=vals16_ext.rearrange("t p h -> p t h"))

        # v embedded at column 32 of a zero pad (host-built); the M=32
        # window [:, 32-g:64-g] puts v at window-column g, so the matvec
        # result lands in row g of a 32-partition PSUM strip.
        sb_vpad = singles.tile([128, 64], t_dtype)
        nc.sync.dma_start(out=sb_vpad, in_=vpad_ext[:])
        identity16 = singles.tile([128, 128], F16)
        nc.sync.dma_start(out=identity16, in_=id_ext[:])

        # --- pqT [u, q] -----------------------------------------------
        sb_pqT = singles.tile([128, QL], F32)
        with tc.tile_pool(name="ps_pq", bufs=1, space="PSUM") as pqpool:
            ps_pq = pqpool.tile([128, QL], F32)
            for ht in range(HT):
                nc.tensor.matmul(
                    ps_pq, lhsT=sb_w1[:, ht, :], rhs=sb_qT[:, ht, :],
                    start=(ht == 0), stop=(ht == HT - 1),
                )
            nc.vector.tensor_copy(out=sb_pqT, in_=ps_pq)

        with tc.tile_pool(name="ps_scores", bufs=1, space="PSUM") as scpool:
            psum_scores = scpool.tile([128, V], F32)

            # --- pvT [u, v] built via PSUM, copied to SBUF (fp16 so the
            # DVE pre-adds hit 4x mode) ---------------------------------
            sb_pvT = singles.tile([128, V], F16)
            with tc.tile_pool(name="ps_pvt", bufs=2, space="PSUM") as pvpool:
                for c in range(NB):
                    ps_pv = pvpool.tile([128, 512], F32, tag="pv")
                    halves = ((0, 256), (256, 512)) if c == 0 else ((0, 512),)
                    for lo, hi in halves:
                        for ht in range(HT):
                            nc.tensor.matmul(
                                ps_pv[:, lo:hi],
                                lhsT=sb_w2[:, ht, :],
                                rhs=sb_valsT[:, c, ht, lo:hi],
                                start=(ht == 0), stop=(ht == HT - 1),
                            )
                        nc.vector.tensor_copy(
                            out=sb_pvT[:, c * 512 + lo:c * 512 + hi],
                            in_=ps_pv[:, lo:hi])

            # --- main loop -------------------------------------------
            # 16 iterations of 8 q's each: two matvec waves (lanes 2j and
            # 2j+1) share one merged ACT instruction [128, 8*2048] to
            # amortize the per-instruction overhead 8x. First and last
            # iterations are chunked per 512 cols to pipeline against the
            # head DMAs / tail softmax.
            for j in range(16):
                addbuf = apool.tile([128, 8, V], F16, tag="add")
                t_t = tpool.tile([128, 8, V], F16, tag="t")
                if j == 0:
                    spans = [(0, 256), (256, 512)] + [
                        (512 * c, 512 * (c + 1)) for c in range(1, NB)]
                    for lo, hi in spans:
                        cs = slice(lo, hi)
                        for b in range(2):
                            for s in range(4):
                                q = 32 * s + 2 * j + b
                                nc.vector.tensor_scalar_add(
                                    addbuf[:, b * 4 + s, cs], sb_pvT[:, cs],
                                    sb_pqT[:, q:q + 1])
                            nc.scalar.activation(
                                out=t_t[:, b * 4:b * 4 + 4, cs],
                                in_=addbuf[:, b * 4:b * 4 + 4, cs],
                                func=mybir.ActivationFunctionType.Tanh,
                            )
                else:
                    for b in range(2):
                        for s in range(4):
                            q = 32 * s + 2 * j + b
                            nc.vector.tensor_scalar_add(
                                addbuf[:, b * 4 + s, :], sb_pvT,
                                sb_pqT[:, q:q + 1])
                    if j == 15:
                        for c in range(NB):
                            cs = slice(c * 512, (c + 1) * 512)
                            nc.scalar.activation(
                                out=t_t[:, :, cs], in_=addbuf[:, :, cs],
                                func=mybir.ActivationFunctionType.Tanh,
                            )
                    else:
                        nc.scalar.activation(
                            out=t_t.rearrange("p s v -> p (s v)"),
                            in_=addbuf.rearrange("p s v -> p (s v)"),
                            func=mybir.ActivationFunctionType.Tanh,
                        )
                for b in range(2):
                    g = 2 * j + b
                    for nb in range(NB):
                        for s in range(4):
                            nc.tensor.matmul(
                                psum_scores[32 * s:32 * s + 32,
                                            nb * 512:(nb + 1) * 512],
                                lhsT=sb_vpad[:, 32 - g:64 - g],
                                rhs=t_t[:, b * 4 + s, nb * 512:(nb + 1) * 512],
                                start=(j == 0 and b == 0),
                                stop=(j == 15 and b == 1),
                                tile_position=(0, 32 * s),
                                skip_group_check=True,
                            )

            # --- softmax + output, overlapped ------------------------
            # Quartered exp (no max-subtract; |scores| <= sum|v| ~ 9) so
            # the eT transposes + output matmuls start after the first
            # quarter; the row-sum runs on DVE under the final matmuls.
            sb_e = singles.tile([128, V], F16)
            sb_sums = work.tile([128, 4], F32)
            with tc.tile_pool(name="ps_out", bufs=1, space="PSUM") as outpool, \
                    tc.tile_pool(name="ps_tr", bufs=3, space="PSUM") as trpool:
                ps_out = outpool.tile([128, H], F32, tag="ps_out")
                for k in range(4):
                    ks = slice(k * 512, (k + 1) * 512)
                    nc.scalar.activation(
                        out=sb_e[:, ks], in_=psum_scores[:, ks],
                        func=mybir.ActivationFunctionType.Exp,
                        bias=0.0, scale=1.0, accum_out=sb_sums[:, k:k + 1],
                    )
                for vt in range(VT):
                    ps_tr = trpool.tile([128, 128], F16, tag="ps_tr")
                    nc.tensor.transpose(
                        ps_tr, sb_e[:, vt * 128:(vt + 1) * 128], identity16)
                    sb_eT_t = work.tile([128, 128], F16, tag="eT")
                    nc.vector.tensor_copy(out=sb_eT_t, in_=ps_tr)
                    nc.tensor.matmul(
                        ps_out, lhsT=sb_eT_t, rhs=sb_vals16[:, vt, :],
                        start=(vt == 0), stop=(vt == VT - 1),
                        skip_group_check=True,
                    )
                sb_sum = work.tile([128, 1], F32)
                nc.vector.tensor_reduce(
                    out=sb_sum, in_=sb_sums, axis=mybir.AxisListType.X,
                    op=mybir.AluOpType.add)
                sb_rsum = work.tile([128, 1], F32)
                nc.vector.reciprocal(sb_rsum, sb_sum)
                sb_out = work.tile([128, H], F32)
                nc.vector.tensor_scalar_mul(sb_out, ps_out, sb_rsum)
                nc.sync.dma_start(out=out_ext[:], in_=sb_out)

    nc.finalize()
    return nc


_NC_CACHE = {}


def _get_nc():
    if "nc" not in _NC_CACHE:
        _NC_CACHE["nc"] = build_nc()
    return _NC_CACHE["nc"]


def make_in_maps(queries, values, w1, w2, v):
    w1s = np.ascontiguousarray(w1, np.float32).reshape(HT, 128, U)
    w2s = np.ascontiguousarray(w2, np.float32).reshape(HT, 128, U)
    vpad = np.zeros((128, 64), np.float16)
    vpad[:, 32] = np.asarray(v, np.float32).astype(np.float16)
    ident = np.eye(128, dtype=np.float16)
    queries = np.asarray(queries, np.float32)
    values = np.asarray(values, np.float32)
    in_maps = []
    for c in range(8):
        b, qh = c // 2, c % 2
        q_shard = queries[b, qh * QL:(qh + 1) * QL, :]        # [QL, H]
        vb = values[b]                                        # [V, H]
        vbT = np.ascontiguousarray(vb.T)                      # [H, V]
        valsT = np.ascontiguousarray(
            vbT.reshape(HT, 128, NB, 512).transpose(2, 0, 1, 3))
        in_maps.append({
            "qT": np.ascontiguousarray(q_shard.T).reshape(HT, 128, QL),
            "valsT": valsT,
            "vals16": np.ascontiguousarray(vb.astype(np.float16)).reshape(VT, 128, H),
            "w1": w1s, "w2": w2s, "vpad": vpad, "identity16": ident,
        })
    return in_maps


def gather_out(results):
    out = np.empty((B, Q, H), np.float32)
    for c in range(8):
        b, qh = c // 2, c % 2
        out[b, qh * QL:(qh + 1) * QL, :] = results[c]["out"]
    return out


def kernel(queries, values, w1, w2, v):
    from concourse.bass_utils import run_bass_kernel_spmd

    nc = _get_nc()
    in_maps = make_in_maps(queries, values, w1, w2, v)
    res = run_bass_kernel_spmd(nc, in_maps, list(range(8)))
    return gather_out(res.results)
